# revision 24
# baseline (speedup 1.0000x reference)
"""Fused multi-head attention (B=4, L=2048, D=1024, H=16) for 8 Trainium2 cores.

Sharding: core c = 2*b + g handles batch b, head-group g (8 heads).
W_q/W_k sliced+row-permuted (RoPE interleave) column-parallel, W_o
row-parallel; host sums the two partial outputs per batch (Megatron-style).

Per-core kernel layout:
- scores are computed TRANSPOSED (S_T[ki, qi]) so softmax(P) @ V needs no
  on-chip transpose of P; softmax runs without max-subtraction (logits
  bounded for this problem's scale); /sqrt(hd) and +mask fold into the Exp
  activation; denominator comes free from a ones-column appended to V.
- RoPE row layout per head: [x0(0:16), x1(0:16), x0(16:32), x1(16:32)] so
  the pair swap is a within-32-partition stream_shuffle; the combine is one
  fused scalar_tensor_tensor with a per-partition sign.
- Schedule sweeps half 0 of all heads, then half 1 with the output
  projection interleaved; v/qk projections fill early PE slack.
"""

import sys
from contextlib import ExitStack

import numpy as np

sys.path.insert(0, "/opt/trn_rl_repo")

import ml_dtypes  # noqa: E402

import concourse.bass as bass  # noqa: E402
import concourse.mybir as mybir  # noqa: E402
import concourse.tile as tile  # noqa: E402
from concourse import bacc  # noqa: E402

BF16 = mybir.dt.bfloat16
F32 = mybir.dt.float32
AF = mybir.ActivationFunctionType
ALU = mybir.AluOpType

B, L, D = 4, 2048, 1024
H, HD = 16, 64
HPC = 8          # heads per core
DH = HPC * HD    # 512 local head dims
NKT = L // 128   # 16 ki tiles
NQB = L // 128   # 16 qi blocks
HALF = 1024      # qi half width


def build_nc(repeats=1):
    nc = bacc.Bacc(
        "TRN2", target_bir_lowering=False, debug=False, enable_asserts=False
    )

    # DRAM I/O (per-core shards, host-prepared layouts)
    xt_d = nc.dram_tensor("xt", [128, 8 * L], BF16, kind="ExternalInput").ap()
    wq_d = nc.dram_tensor("wq", [128, 8 * DH], BF16, kind="ExternalInput").ap()
    wk_d = nc.dram_tensor("wk", [128, 8 * DH], BF16, kind="ExternalInput").ap()
    wv_d = nc.dram_tensor("wv", [128, 8 * DH], BF16, kind="ExternalInput").ap()
    wo_d = nc.dram_tensor("wo", [128, 4 * D], BF16, kind="ExternalInput").ap()
    cos_d = nc.dram_tensor("cosT", [128, L], BF16, kind="ExternalInput").ap()
    sin_d = nc.dram_tensor("sinT", [128, L], BF16, kind="ExternalInput").ap()
    mask_d = nc.dram_tensor("maskT", [128, NKT], F32, kind="ExternalInput").ap()
    out_d = nc.dram_tensor("out", [L, D], BF16, kind="ExternalOutput").ap()

    with tile.TileContext(nc) as tc, ExitStack() as ctx:
        io = ctx.enter_context(tc.tile_pool(name="io", bufs=1))
        rp = ctx.enter_context(tc.tile_pool(name="rp", bufs=2))
        esp = ctx.enter_context(tc.tile_pool(name="esp", bufs=18))
        mis = ctx.enter_context(tc.tile_pool(name="mis", bufs=4))
        obp = ctx.enter_context(tc.tile_pool(name="obp", bufs=3))
        pp = ctx.enter_context(tc.tile_pool(name="pp", bufs=2, space="PSUM"))
        prp = ctx.enter_context(tc.tile_pool(name="prp", bufs=1, space="PSUM"))
        otp = ctx.enter_context(tc.tile_pool(name="otp", bufs=2, space="PSUM"))

        # ---- load inputs (order matters: first-needed first) ----
        maskT = io.tile([128, NKT], F32)
        nc.sync.dma_start(maskT[:], mask_d)
        wq = io.tile([128, 8 * DH], BF16)
        nc.sync.dma_start(wq[:], wq_d)
        # xt in 4 chunks (k-tile pairs) so the first projections start early
        xt_c = []
        for i in range(4):
            t = io.tile([128, 2 * L], BF16, name=f"xt{i}")
            nc.sync.dma_start(t[:], xt_d[:, 2 * L * i : 2 * L * (i + 1)])
            xt_c.append(t)
        wk = io.tile([128, 8 * DH], BF16)
        nc.sync.dma_start(wk[:], wk_d)
        cosT = io.tile([128, L], BF16)
        nc.sync.dma_start(cosT[:], cos_d)
        sinT = io.tile([128, L], BF16)
        nc.sync.dma_start(sinT[:], sin_d)
        wv = io.tile([128, 8 * DH], BF16)
        nc.sync.dma_start(wv[:], wv_d)
        wo = io.tile([128, 4 * D], BF16)
        nc.sync.dma_start(wo[:], wo_d)

        def xt_at(k, off, width):
            """x k-tile k, columns [off, off+width) of 2048."""
            return xt_c[k // 2][:, 2048 * (k % 2) + off :][:, :width]

        ones64 = io.tile([1, 64], F32)
        nc.vector.memset(ones64[:], 1.0)
        # rope sign: rows (r%32)<16 -> -1 else +1
        sign = io.tile([128, 1], F32)
        nc.vector.memset(sign[:], 1.0)
        for blk in range(4):
            nc.vector.memset(sign[32 * blk : 32 * blk + 16, :], -1.0)
        # shuffle mask: swap 16-partition halves within each 32 block
        SHUF = [(i + 16) % 32 for i in range(32)]

        # persistent SBUF activations (split by qi/ki half to avoid false deps)
        q_sb = [[io.tile([128, HALF], BF16, name=f"q{m}_{hf}") for hf in range(2)]
                for m in range(4)]
        k_sb = [[io.tile([128, HALF], BF16, name=f"k{m}_{hf}") for hf in range(2)]
                for m in range(4)]
        o_sb = [[io.tile([128, HALF], BF16, name=f"o{m}_{hf}") for hf in range(2)]
                for m in range(4)]
        v_sb = [io.tile([128, HPC * 65], BF16, name=f"v_sb{t}") for t in range(NKT)]
        for t in range(NKT):
            v3 = v_sb[t][:].rearrange("p (h c) -> p h c", c=65)
            nc.vector.memset(v3[:, :, 64:65], 1.0)

        def qk_proj(m, half, w_sb, dst_tiles):
            """project m-tile (heads 2m,2m+1), qi/ki half -> rope -> bf16."""
            dst = dst_tiles[m][half]
            ps = prp.tile([128, HALF], F32, tag="pj", name="ps_proj")
            for k in range(8):
                lhsT = w_sb[:, 512 * k + 128 * m : 512 * k + 128 * m + 128]
                for c in range(2):
                    nc.tensor.matmul(
                        ps[:, 512 * c : 512 * (c + 1)],
                        lhsT,
                        xt_at(k, HALF * half + 512 * c, 512),
                        start=(k == 0),
                        stop=(k == 7),
                    )
            hs = slice(HALF * half, HALF * (half + 1))
            p1 = rp.tile([128, HALF], BF16, tag="p1")
            nc.vector.tensor_mul(p1[:], ps[:], cosT[:, hs])
            p2 = rp.tile([128, HALF], BF16, tag="p2")
            nc.vector.tensor_mul(p2[:], ps[:], sinT[:, hs])
            q2 = rp.tile([128, HALF], BF16, tag="q2")
            nc.vector.stream_shuffle(q2[:], p2[:], SHUF)
            nc.vector.scalar_tensor_tensor(
                dst[:], q2[:], sign[:], p1[:], ALU.mult, ALU.add
            )

        def v_proj(kb):
            ps_v = prp.tile([128, DH], F32, tag="pj", name="ps_v")
            for k in range(8):
                nc.tensor.matmul(
                    ps_v[:],
                    xt_at(k, 128 * kb, 128),
                    wv[:, 512 * k : 512 * (k + 1)],
                    start=(k == 0),
                    stop=(k == 7),
                )
            v3 = v_sb[kb][:].rearrange("p (h c) -> p h c", c=65)
            nc.vector.tensor_copy(
                v3[:, :, 0:64], ps_v[:].rearrange("p (h c) -> p h c", c=64)
            )

        def attn_scores(h, half, t):
            """one score tile + exp; returns the es tile."""
            m, o = h // 2, 64 * (h % 2)
            kt = k_sb[m][t // 8]
            ko = 128 * (t % 8)
            st = pp.tile([128, HALF], F32, tag="st", name="ps_st")
            for c in range(2):
                nc.tensor.matmul(
                    st[:, 512 * c : 512 * (c + 1)],
                    kt[o : o + 64, ko : ko + 128],
                    q_sb[m][half][o : o + 64, 512 * c : 512 * (c + 1)],
                    start=True,
                    stop=True,
                )
            es = esp.tile([128, HALF], BF16, tag="es")
            nc.scalar.activation(
                es[:], st[:], AF.Exp,
                bias=maskT[:, t : t + 1], scale=0.125,
            )
            return es

        def attn_pv(h, otc, t, es):
            for c in range(2):
                nc.tensor.matmul(
                    otc[c][:],
                    v_sb[t][:, 65 * h : 65 * h + 65],
                    es[:, 512 * c : 512 * (c + 1)],
                    start=(t == 0),
                    stop=(t == NKT - 1),
                )

        def attn_epilogue(h, half, otc):
            m, o = h // 2, 64 * (h % 2)
            for c in range(2):
                rec = mis.tile([1, 512], F32, tag="rec")
                nc.vector.reciprocal(rec[:], otc[c][64:65, :])
                bc = pp.tile([64, 512], F32, tag="st", name="ps_bc")
                nc.tensor.matmul(bc[:], ones64[:], rec[0:1, :],
                                 start=True, stop=True)
                bcs = mis.tile([64, 512], F32, tag="bcs")
                nc.vector.tensor_copy(bcs[:], bc[:])
                nc.vector.scalar_tensor_tensor(
                    o_sb[m][half][o : o + 64, 512 * c : 512 * (c + 1)],
                    otc[c][0:64, :], 1.0, bcs[:], ALU.mult, ALU.mult,
                )

        def attn_phase(cur, prev, fillers=()):
            """Pipeline phase: slot `cur`=(h,half) scores+exp, interleaved
            with slot `prev`'s PVs (from its saved es tiles) + fillers.
            Returns cur's state for the next phase."""
            ess = []
            if prev is not None:
                ph, phalf, pess = prev
                potc = [
                    otp.tile([65, 512], F32, tag="ot", name=f"ot{c}")
                    for c in range(2)
                ]
            fillers = dict(fillers)
            for t in range(NKT):
                if cur is not None:
                    ess.append(attn_scores(cur[0], cur[1], t))
                if t in fillers:
                    fillers[t]()
                if prev is not None:
                    attn_pv(ph, potc, t, pess[t])
            if prev is not None:
                attn_epilogue(ph, phalf, potc)
            if cur is None:
                return None
            return (cur[0], cur[1], ess)

        def outproj_qb(qb, tags=("pj", "pj")):
            hf, qo = qb // 8, 128 * (qb % 8)
            for c in range(2):
                pool = pp if tags[c] == "st" else prp
                po = pool.tile([128, 512], F32, tag=tags[c], name="ps_po")
                for dt_ in range(4):
                    nc.tensor.matmul(
                        po[:],
                        o_sb[dt_][hf][:, qo : qo + 128],
                        wo[:, D * dt_ + 512 * c : D * dt_ + 512 * (c + 1)],
                        start=(dt_ == 0),
                        stop=(dt_ == 3),
                    )
                ob = obp.tile([128, 512], BF16, tag="ob")
                nc.vector.tensor_copy(ob[:], po[:])
                nc.sync.dma_start(
                    out_d[128 * qb : 128 * (qb + 1), 512 * c : 512 * (c + 1)],
                    ob[:],
                )

        def qk_group(m):
            qk_proj(m, 0, wq, q_sb)
            qk_proj(m, 0, wk, k_sb)
            qk_proj(m, 1, wk, k_sb)

        for _rep in range(repeats):
            qk_group(0)
            s = attn_phase(
                (0, 0), None,
                fillers={t: (lambda kb=t: v_proj(kb)) for t in range(NKT)},
            )
            qk_group(1)
            s = attn_phase((1, 0), s)
            qk_group(2)
            s = attn_phase((2, 0), s)
            qk_group(3)
            s = attn_phase((3, 0), s)
            s = attn_phase((4, 0), s)
            s = attn_phase((5, 0), s)
            qk_proj(0, 1, wq, q_sb)
            s = attn_phase((6, 0), s)
            qk_proj(1, 1, wq, q_sb)
            s = attn_phase((7, 0), s)
            qk_proj(2, 1, wq, q_sb)
            s = attn_phase((0, 1), s)
            qk_proj(3, 1, wq, q_sb)
            # half-1 phases; outproj of half-0 qi blocks as fillers
            qbf = [[0], [1], [2], [3], [4, 5], [6, 7]]
            for i, h in enumerate(range(1, 7)):
                fl = {
                    4 + 8 * j: (lambda qb=qb: outproj_qb(qb))
                    for j, qb in enumerate(qbf[i])
                }
                s = attn_phase((h, 1), s, fillers=fl)
            s = attn_phase((7, 1), s)
            attn_phase(None, s)
            for qb in range(8, NQB):
                outproj_qb(qb, tags=("st", "pj"))
    nc.compile()
    return nc


def _prep_core_inputs(x, cosT, sinT, mask, W_q, W_k, W_v, W_o, b, g):
    bf = ml_dtypes.bfloat16
    gs = slice(g * DH, (g + 1) * DH)

    # RoPE interleave row permutation within the head-group slice:
    # per head: [x0(0:16), x1(0:16), x0(16:32), x1(16:32)]
    j = np.arange(64)
    blk, within = j // 16, j % 16
    perm64 = np.where(
        blk == 0, 2 * within,
        np.where(blk == 1, 2 * within + 1,
                 np.where(blk == 2, 2 * within + 32, 2 * within + 33)),
    )
    perm = (np.arange(HPC)[:, None] * 64 + perm64[None, :]).reshape(-1) + g * DH

    def wtile(wT):  # [1024, 512] -> [128, 8*512] (k-tile k at cols 512k)
        return np.ascontiguousarray(
            wT.reshape(8, 128, DH).transpose(1, 0, 2).reshape(128, 8 * DH)
        ).astype(bf)

    xt = np.ascontiguousarray(
        x[b].T.reshape(8, 128, L).transpose(1, 0, 2).reshape(128, 8 * L)
    ).astype(bf)
    wq = wtile(W_q[perm].T)
    wk = wtile(W_k[perm].T)
    wv = wtile(W_v[gs].T)
    wo = np.ascontiguousarray(
        W_o[:, gs].T.reshape(4, 128, D).transpose(1, 0, 2).reshape(128, 4 * D)
    ).astype(bf)
    return {
        "xt": xt, "wq": wq, "wk": wk, "wv": wv, "wo": wo,
        "cosT": cosT, "sinT": sinT,
        "maskT": np.ascontiguousarray(mask[b].reshape(NKT, 128).T).astype(
            np.float32
        ),
    }


def make_in_maps(x, freqs_cos, freqs_sin, attention_mask, W_q, W_k, W_v, W_o):
    bf = ml_dtypes.bfloat16
    x = np.asarray(x, np.float32)
    # cos/sin rows follow the rope row layout: r%64 -> dim (r%16) + 16*((r%64)//32)
    r = np.arange(128)
    dim_idx = (r % 16) + 16 * ((r % 64) // 32)
    cosT = np.ascontiguousarray(
        np.asarray(freqs_cos, np.float32).T[dim_idx]
    ).astype(bf)
    sinT = np.ascontiguousarray(
        np.asarray(freqs_sin, np.float32).T[dim_idx]
    ).astype(bf)
    mask = np.asarray(attention_mask, np.float32)
    W_q, W_k = np.asarray(W_q, np.float32), np.asarray(W_k, np.float32)
    W_v, W_o = np.asarray(W_v, np.float32), np.asarray(W_o, np.float32)
    return [
        _prep_core_inputs(x, cosT, sinT, mask, W_q, W_k, W_v, W_o, c // 2, c % 2)
        for c in range(8)
    ]


_CACHE = {}


def kernel(x, freqs_cos, freqs_sin, attention_mask, W_q, W_k, W_v, W_o):
    from concourse.bass_utils import run_bass_kernel_spmd

    if "nc" not in _CACHE:
        _CACHE["nc"] = build_nc()
    nc = _CACHE["nc"]
    in_maps = make_in_maps(
        x, freqs_cos, freqs_sin, attention_mask, W_q, W_k, W_v, W_o
    )
    res = run_bass_kernel_spmd(nc, in_maps, core_ids=list(range(8)))
    outs = [np.asarray(r["out"], np.float32) for r in res.results]
    full = np.stack([outs[2 * b] + outs[2 * b + 1] for b in range(B)], axis=0)
    return full.astype(np.float32)


if __name__ == "__main__":
    nc = build_nc()
    print("built ok")


# revision 27
# speedup vs baseline: 1.1951x; 1.1951x over previous
"""Fused multi-head attention (B=4, L=2048, D=1024, H=16) for 8 Trainium2 cores.

Sharding: core c = 2*b + g handles batch b, head-group g (8 heads).
W_q/W_k sliced+row-permuted (RoPE interleave) column-parallel, W_o
row-parallel; host sums the two partial outputs per batch (Megatron-style).

Per-core kernel layout:
- scores are computed TRANSPOSED (S_T[ki, qi]) so softmax(P) @ V needs no
  on-chip transpose of P; softmax runs without max-subtraction (logits
  bounded for this problem's scale); /sqrt(hd) and +mask fold into the Exp
  activation; denominator comes free from a ones-column appended to V.
- RoPE row layout per head: [x0(0:16), x1(0:16), x0(16:32), x1(16:32)] so
  the pair swap is a within-32-partition stream_shuffle; the combine is one
  fused scalar_tensor_tensor with a per-partition sign.
- Schedule sweeps half 0 of all heads, then half 1 with the output
  projection interleaved; v/qk projections fill early PE slack.
"""

import sys
from contextlib import ExitStack

import numpy as np

sys.path.insert(0, "/opt/trn_rl_repo")

import ml_dtypes  # noqa: E402

import concourse.bass as bass  # noqa: E402
import concourse.mybir as mybir  # noqa: E402
import concourse.tile as tile  # noqa: E402
from concourse import bacc, library_config  # noqa: E402

BF16 = mybir.dt.bfloat16
F32 = mybir.dt.float32
AF = mybir.ActivationFunctionType
ALU = mybir.AluOpType

B, L, D = 4, 2048, 1024
H, HD = 16, 64
HPC = 8          # heads per core
DH = HPC * HD    # 512 local head dims
NKT = L // 128   # 16 ki tiles
NQB = L // 128   # 16 qi blocks
HALF = 1024      # qi half width


def build_nc(repeats=1):
    nc = bacc.Bacc(
        "TRN2", target_bir_lowering=False, debug=False, enable_asserts=False
    )

    # DRAM I/O (per-core shards, host-prepared layouts)
    xt_d = nc.dram_tensor("xt", [128, 8 * L], BF16, kind="ExternalInput").ap()
    wq_d = nc.dram_tensor("wq", [128, 8 * DH], BF16, kind="ExternalInput").ap()
    wk_d = nc.dram_tensor("wk", [128, 8 * DH], BF16, kind="ExternalInput").ap()
    wv_d = nc.dram_tensor("wv", [128, 8 * DH], BF16, kind="ExternalInput").ap()
    wo_d = nc.dram_tensor("wo", [128, 4 * D], BF16, kind="ExternalInput").ap()
    cos_d = nc.dram_tensor("cosT", [128, L], BF16, kind="ExternalInput").ap()
    sin_d = nc.dram_tensor("sinT", [128, L], BF16, kind="ExternalInput").ap()
    mask_d = nc.dram_tensor("maskT", [128, NKT], F32, kind="ExternalInput").ap()
    out_d = nc.dram_tensor("out", [L, D], BF16, kind="ExternalOutput").ap()

    with tile.TileContext(nc) as tc, ExitStack() as ctx:
        io = ctx.enter_context(tc.tile_pool(name="io", bufs=1))
        rp = ctx.enter_context(tc.tile_pool(name="rp", bufs=2))
        esp = ctx.enter_context(tc.tile_pool(name="esp", bufs=18))
        mis = ctx.enter_context(tc.tile_pool(name="mis", bufs=4))
        obp = ctx.enter_context(tc.tile_pool(name="obp", bufs=3))
        pp = ctx.enter_context(tc.tile_pool(name="pp", bufs=2, space="PSUM"))
        prp = ctx.enter_context(tc.tile_pool(name="prp", bufs=1, space="PSUM"))
        otp = ctx.enter_context(tc.tile_pool(name="otp", bufs=2, space="PSUM"))

        # ---- load inputs (order matters: first-needed first) ----
        maskT = io.tile([128, NKT], F32)
        nc.sync.dma_start(maskT[:], mask_d)
        wq = io.tile([128, 8 * DH], BF16)
        nc.sync.dma_start(wq[:], wq_d)
        # xt in 4 chunks (k-tile pairs) so the first projections start early
        xt_c = []
        for i in range(4):
            t = io.tile([128, 2 * L], BF16, name=f"xt{i}")
            nc.sync.dma_start(t[:], xt_d[:, 2 * L * i : 2 * L * (i + 1)])
            xt_c.append(t)
        wk = io.tile([128, 8 * DH], BF16)
        nc.sync.dma_start(wk[:], wk_d)
        cosT = io.tile([128, L], BF16)
        nc.sync.dma_start(cosT[:], cos_d)
        sinT = io.tile([128, L], BF16)
        nc.sync.dma_start(sinT[:], sin_d)
        wv = io.tile([128, 8 * DH], BF16)
        nc.sync.dma_start(wv[:], wv_d)
        wo = io.tile([128, 4 * D], BF16)
        nc.sync.dma_start(wo[:], wo_d)

        def xt_at(k, off, width):
            """x k-tile k, columns [off, off+width) of 2048."""
            return xt_c[k // 2][:, 2048 * (k % 2) + off :][:, :width]

        nc.gpsimd.load_library(library_config.attn)
        # rope sign: rows (r%32)<16 -> -1 else +1
        sign = io.tile([128, 1], F32)
        nc.vector.memset(sign[:], 1.0)
        for blk in range(4):
            nc.vector.memset(sign[32 * blk : 32 * blk + 16, :], -1.0)
        # shuffle mask: swap 16-partition halves within each 32 block
        SHUF = [(i + 16) % 32 for i in range(32)]

        # persistent SBUF activations (split by qi/ki half to avoid false deps)
        q_sb = [[io.tile([128, HALF], BF16, name=f"q{m}_{hf}") for hf in range(2)]
                for m in range(4)]
        k_sb = [[io.tile([128, HALF], BF16, name=f"k{m}_{hf}") for hf in range(2)]
                for m in range(4)]
        o_sb = [[io.tile([128, HALF], BF16, name=f"o{m}_{hf}") for hf in range(2)]
                for m in range(4)]
        v_sb = [io.tile([128, HPC * 65], BF16, name=f"v_sb{t}") for t in range(NKT)]
        for t in range(NKT):
            v3 = v_sb[t][:].rearrange("p (h c) -> p h c", c=65)
            nc.vector.memset(v3[:, :, 64:65], 1.0)

        def qk_proj(m, half, w_sb, dst_tiles):
            """project m-tile (heads 2m,2m+1), qi/ki half -> rope -> bf16."""
            dst = dst_tiles[m][half]
            ps = prp.tile([128, HALF], F32, tag="pj", name="ps_proj")
            for k in range(8):
                lhsT = w_sb[:, 512 * k + 128 * m : 512 * k + 128 * m + 128]
                for c in range(2):
                    nc.tensor.matmul(
                        ps[:, 512 * c : 512 * (c + 1)],
                        lhsT,
                        xt_at(k, HALF * half + 512 * c, 512),
                        start=(k == 0),
                        stop=(k == 7),
                    )
            hs = slice(HALF * half, HALF * (half + 1))
            p1 = rp.tile([128, HALF], BF16, tag="p1")
            nc.vector.tensor_mul(p1[:], ps[:], cosT[:, hs])
            p2 = rp.tile([128, HALF], BF16, tag="p2")
            nc.vector.tensor_mul(p2[:], ps[:], sinT[:, hs])
            q2 = rp.tile([128, HALF], BF16, tag="q2")
            nc.vector.stream_shuffle(q2[:], p2[:], SHUF)
            nc.vector.scalar_tensor_tensor(
                dst[:], q2[:], sign[:], p1[:], ALU.mult, ALU.add
            )

        def v_proj(kb):
            ps_v = prp.tile([128, DH], F32, tag="pj", name="ps_v")
            for k in range(8):
                nc.tensor.matmul(
                    ps_v[:],
                    xt_at(k, 128 * kb, 128),
                    wv[:, 512 * k : 512 * (k + 1)],
                    start=(k == 0),
                    stop=(k == 7),
                )
            v3 = v_sb[kb][:].rearrange("p (h c) -> p h c", c=65)
            nc.vector.tensor_copy(
                v3[:, :, 0:64], ps_v[:].rearrange("p (h c) -> p h c", c=64)
            )

        def attn_scores(h, half, t):
            """one score tile + exp; returns the es tile."""
            m, o = h // 2, 64 * (h % 2)
            kt = k_sb[m][t // 8]
            ko = 128 * (t % 8)
            st = pp.tile([128, HALF], F32, tag="st", name="ps_st")
            for c in range(2):
                nc.tensor.matmul(
                    st[:, 512 * c : 512 * (c + 1)],
                    kt[o : o + 64, ko : ko + 128],
                    q_sb[m][half][o : o + 64, 512 * c : 512 * (c + 1)],
                    start=True,
                    stop=True,
                )
            es = esp.tile([128, HALF], BF16, tag="es")
            nc.scalar.activation(
                es[:], st[:], AF.Exp,
                bias=maskT[:, t : t + 1], scale=0.125,
            )
            return es

        def attn_pv(h, otc, t, es):
            for c in range(2):
                nc.tensor.matmul(
                    otc[c][:],
                    v_sb[t][:, 65 * h : 65 * h + 65],
                    es[:, 512 * c : 512 * (c + 1)],
                    start=(t == 0),
                    stop=(t == NKT - 1),
                )

        def attn_epilogue(h, half, otc):
            m, o = h // 2, 64 * (h % 2)
            for c in range(2):
                rec = mis.tile([1, 512], F32, tag="rec")
                nc.vector.reciprocal(rec[:], otc[c][64:65, :])
                bcs = mis.tile([64, 512], F32, tag="bcs")
                nc.gpsimd.partition_broadcast(bcs[:], rec[0:1, :], channels=64)
                nc.vector.scalar_tensor_tensor(
                    o_sb[m][half][o : o + 64, 512 * c : 512 * (c + 1)],
                    otc[c][0:64, :], 1.0, bcs[:], ALU.mult, ALU.mult,
                )

        def attn_phase(cur, prev, fillers=()):
            """Pipeline phase: slot `cur`=(h,half) scores+exp, interleaved
            with slot `prev`'s PVs (from its saved es tiles) + fillers.
            Returns cur's state for the next phase."""
            ess = []
            if prev is not None:
                ph, phalf, pess = prev
                potc = [
                    otp.tile([65, 512], F32, tag="ot", name=f"ot{c}")
                    for c in range(2)
                ]
            fillers = dict(fillers)
            for t in range(NKT):
                if cur is not None:
                    ess.append(attn_scores(cur[0], cur[1], t))
                if t in fillers:
                    fillers[t]()
                if prev is not None:
                    attn_pv(ph, potc, t, pess[t])
            if prev is not None:
                attn_epilogue(ph, phalf, potc)
            if cur is None:
                return None
            return (cur[0], cur[1], ess)

        def outproj_qb(qb, tags=("pj", "pj")):
            hf, qo = qb // 8, 128 * (qb % 8)
            for c in range(2):
                pool = pp if tags[c] == "st" else prp
                po = pool.tile([128, 512], F32, tag=tags[c], name="ps_po")
                for dt_ in range(4):
                    nc.tensor.matmul(
                        po[:],
                        o_sb[dt_][hf][:, qo : qo + 128],
                        wo[:, D * dt_ + 512 * c : D * dt_ + 512 * (c + 1)],
                        start=(dt_ == 0),
                        stop=(dt_ == 3),
                    )
                ob = obp.tile([128, 512], BF16, tag="ob")
                nc.vector.tensor_copy(ob[:], po[:])
                nc.sync.dma_start(
                    out_d[128 * qb : 128 * (qb + 1), 512 * c : 512 * (c + 1)],
                    ob[:],
                )

        def qk_group(m):
            qk_proj(m, 0, wq, q_sb)
            qk_proj(m, 0, wk, k_sb)
            qk_proj(m, 1, wk, k_sb)

        for _rep in range(repeats):
            qk_group(0)
            s = attn_phase(
                (0, 0), None,
                fillers={t: (lambda kb=t: v_proj(kb)) for t in range(NKT)},
            )
            qk_group(1)
            s = attn_phase((1, 0), s)
            qk_group(2)
            s = attn_phase((2, 0), s)
            qk_group(3)
            s = attn_phase((3, 0), s)
            s = attn_phase((4, 0), s)
            s = attn_phase((5, 0), s)
            qk_proj(0, 1, wq, q_sb)
            s = attn_phase((6, 0), s)
            qk_proj(1, 1, wq, q_sb)
            s = attn_phase((7, 0), s)
            qk_proj(2, 1, wq, q_sb)
            s = attn_phase((0, 1), s)
            qk_proj(3, 1, wq, q_sb)
            # half-1 phases; outproj of half-0 qi blocks as fillers
            qbf = [[0], [1], [2], [3], [4, 5], [6, 7]]
            for i, h in enumerate(range(1, 7)):
                fl = {
                    4 + 8 * j: (lambda qb=qb: outproj_qb(qb))
                    for j, qb in enumerate(qbf[i])
                }
                s = attn_phase((h, 1), s, fillers=fl)
            s = attn_phase((7, 1), s)
            attn_phase(None, s)
            for qb in range(8, NQB):
                outproj_qb(qb, tags=("st", "pj"))
    nc.compile()
    return nc


def _prep_core_inputs(x, cosT, sinT, mask, W_q, W_k, W_v, W_o, b, g):
    bf = ml_dtypes.bfloat16
    gs = slice(g * DH, (g + 1) * DH)

    # RoPE interleave row permutation within the head-group slice:
    # per head: [x0(0:16), x1(0:16), x0(16:32), x1(16:32)]
    j = np.arange(64)
    blk, within = j // 16, j % 16
    perm64 = np.where(
        blk == 0, 2 * within,
        np.where(blk == 1, 2 * within + 1,
                 np.where(blk == 2, 2 * within + 32, 2 * within + 33)),
    )
    perm = (np.arange(HPC)[:, None] * 64 + perm64[None, :]).reshape(-1) + g * DH

    def wtile(wT):  # [1024, 512] -> [128, 8*512] (k-tile k at cols 512k)
        return np.ascontiguousarray(
            wT.reshape(8, 128, DH).transpose(1, 0, 2).reshape(128, 8 * DH)
        ).astype(bf)

    xt = np.ascontiguousarray(
        x[b].T.reshape(8, 128, L).transpose(1, 0, 2).reshape(128, 8 * L)
    ).astype(bf)
    wq = wtile(W_q[perm].T)
    wk = wtile(W_k[perm].T)
    wv = wtile(W_v[gs].T)
    wo = np.ascontiguousarray(
        W_o[:, gs].T.reshape(4, 128, D).transpose(1, 0, 2).reshape(128, 4 * D)
    ).astype(bf)
    return {
        "xt": xt, "wq": wq, "wk": wk, "wv": wv, "wo": wo,
        "cosT": cosT, "sinT": sinT,
        "maskT": np.ascontiguousarray(mask[b].reshape(NKT, 128).T).astype(
            np.float32
        ),
    }


def make_in_maps(x, freqs_cos, freqs_sin, attention_mask, W_q, W_k, W_v, W_o):
    bf = ml_dtypes.bfloat16
    x = np.asarray(x, np.float32)
    # cos/sin rows follow the rope row layout: r%64 -> dim (r%16) + 16*((r%64)//32)
    r = np.arange(128)
    dim_idx = (r % 16) + 16 * ((r % 64) // 32)
    cosT = np.ascontiguousarray(
        np.asarray(freqs_cos, np.float32).T[dim_idx]
    ).astype(bf)
    sinT = np.ascontiguousarray(
        np.asarray(freqs_sin, np.float32).T[dim_idx]
    ).astype(bf)
    mask = np.asarray(attention_mask, np.float32)
    W_q, W_k = np.asarray(W_q, np.float32), np.asarray(W_k, np.float32)
    W_v, W_o = np.asarray(W_v, np.float32), np.asarray(W_o, np.float32)
    return [
        _prep_core_inputs(x, cosT, sinT, mask, W_q, W_k, W_v, W_o, c // 2, c % 2)
        for c in range(8)
    ]


_CACHE = {}


def kernel(x, freqs_cos, freqs_sin, attention_mask, W_q, W_k, W_v, W_o):
    from concourse.bass_utils import run_bass_kernel_spmd

    if "nc" not in _CACHE:
        _CACHE["nc"] = build_nc()
    nc = _CACHE["nc"]
    in_maps = make_in_maps(
        x, freqs_cos, freqs_sin, attention_mask, W_q, W_k, W_v, W_o
    )
    res = run_bass_kernel_spmd(nc, in_maps, core_ids=list(range(8)))
    outs = [np.asarray(r["out"], np.float32) for r in res.results]
    full = np.stack([outs[2 * b] + outs[2 * b + 1] for b in range(B)], axis=0)
    return full.astype(np.float32)


if __name__ == "__main__":
    nc = build_nc()
    print("built ok")


# revision 38
# speedup vs baseline: 1.2266x; 1.0264x over previous
"""Fused multi-head attention (B=4, L=2048, D=1024, H=16) for 8 Trainium2 cores.

Sharding: core c = 2*b + g handles batch b, head-group g (8 heads).
W_q/W_k sliced+row-permuted (RoPE interleave) column-parallel, W_o
row-parallel; host sums the two partial outputs per batch (Megatron-style).

Per-core kernel layout:
- scores are computed TRANSPOSED (S_T[ki, qi]) so softmax(P) @ V needs no
  on-chip transpose of P; softmax runs without max-subtraction (logits
  bounded for this problem's scale); /sqrt(hd) and +mask fold into the Exp
  activation; denominator comes free from a ones-column appended to V.
- RoPE row layout per head: [x0(0:16), x1(0:16), x0(16:32), x1(16:32)] so
  the pair swap is a within-32-partition stream_shuffle; the combine is one
  fused scalar_tensor_tensor with a per-partition sign.
- Schedule sweeps half 0 of all heads, then half 1 with the output
  projection interleaved; v/qk projections fill early PE slack.
"""

import sys
from contextlib import ExitStack

import numpy as np

sys.path.insert(0, "/opt/trn_rl_repo")

import ml_dtypes  # noqa: E402

import concourse.bass as bass  # noqa: E402
import concourse.mybir as mybir  # noqa: E402
import concourse.tile as tile  # noqa: E402
from concourse import bacc, library_config  # noqa: E402

BF16 = mybir.dt.bfloat16
F32 = mybir.dt.float32
AF = mybir.ActivationFunctionType
ALU = mybir.AluOpType

B, L, D = 4, 2048, 1024
H, HD = 16, 64
HPC = 8          # heads per core
DH = HPC * HD    # 512 local head dims
NKT = L // 128   # 16 ki tiles
NQB = L // 128   # 16 qi blocks
HALF = 1024      # qi half width


def build_nc(repeats=1, variant="cur"):
    nc = bacc.Bacc(
        "TRN2", target_bir_lowering=False, debug=False, enable_asserts=False
    )

    # DRAM I/O (per-core shards, host-prepared layouts)
    xt_d = nc.dram_tensor("xt", [128, 8 * L], BF16, kind="ExternalInput").ap()
    wq_d = nc.dram_tensor("wq", [128, 8 * DH], BF16, kind="ExternalInput").ap()
    wk_d = nc.dram_tensor("wk", [128, 8 * DH], BF16, kind="ExternalInput").ap()
    wv_d = nc.dram_tensor("wv", [128, 8 * DH], BF16, kind="ExternalInput").ap()
    wo_d = nc.dram_tensor("wo", [128, 4 * D], BF16, kind="ExternalInput").ap()
    cos_d = nc.dram_tensor("cosT", [128, L], BF16, kind="ExternalInput").ap()
    sin_d = nc.dram_tensor("sinT", [128, L], BF16, kind="ExternalInput").ap()
    mask_d = nc.dram_tensor("maskT", [128, NKT], F32, kind="ExternalInput").ap()
    out_d = nc.dram_tensor("out", [L, D], BF16, kind="ExternalOutput").ap()

    with tile.TileContext(nc) as tc, ExitStack() as ctx:
        io = ctx.enter_context(tc.tile_pool(name="io", bufs=1))
        rp = ctx.enter_context(tc.tile_pool(name="rp", bufs=2))
        esp = ctx.enter_context(tc.tile_pool(name="esp", bufs=20))
        mis = ctx.enter_context(tc.tile_pool(name="mis", bufs=4))
        obp = ctx.enter_context(tc.tile_pool(name="obp", bufs=3))
        pp = ctx.enter_context(tc.tile_pool(name="pp", bufs=2, space="PSUM"))
        prp = ctx.enter_context(tc.tile_pool(name="prp", bufs=2, space="PSUM"))
        otp = ctx.enter_context(tc.tile_pool(name="otp", bufs=2, space="PSUM"))

        # ---- load inputs (order matters: first-needed first) ----
        maskT = io.tile([128, NKT], F32)
        nc.sync.dma_start(maskT[:], mask_d)
        wq = io.tile([128, 8 * DH], BF16)
        nc.sync.dma_start(wq[:], wq_d)
        # xt in 4 chunks (k-tile pairs) so the first projections start early
        xt_c = []
        for i in range(4):
            t = io.tile([128, 2 * L], BF16, name=f"xt{i}")
            nc.sync.dma_start(t[:], xt_d[:, 2 * L * i : 2 * L * (i + 1)])
            xt_c.append(t)
        wk = io.tile([128, 8 * DH], BF16)
        nc.sync.dma_start(wk[:], wk_d)
        cosT = io.tile([128, L], BF16)
        nc.sync.dma_start(cosT[:], cos_d)
        sinT = io.tile([128, L], BF16)
        nc.sync.dma_start(sinT[:], sin_d)
        wv = io.tile([128, 8 * DH], BF16)
        nc.sync.dma_start(wv[:], wv_d)
        wo = io.tile([128, 4 * D], BF16)
        nc.sync.dma_start(wo[:], wo_d)

        def xt_at(k, off, width):
            """x k-tile k, columns [off, off+width) of 2048."""
            return xt_c[k // 2][:, 2048 * (k % 2) + off :][:, :width]

        nc.gpsimd.load_library(library_config.attn)
        # rope sign: rows (r%32)<16 -> -1 else +1
        sign = io.tile([128, 1], F32)
        nc.vector.memset(sign[:], 1.0)
        for blk in range(4):
            nc.vector.memset(sign[32 * blk : 32 * blk + 16, :], -1.0)
        # shuffle mask: swap 16-partition halves within each 32 block
        SHUF = [(i + 16) % 32 for i in range(32)]

        # persistent SBUF activations (split by qi/ki half to avoid false deps)
        q_sb = [[io.tile([128, HALF], BF16, name=f"q{m}_{hf}") for hf in range(2)]
                for m in range(4)]
        k_sb = [[io.tile([128, HALF], BF16, name=f"k{m}_{hf}") for hf in range(2)]
                for m in range(4)]
        o_sb = [[io.tile([128, HALF], BF16, name=f"o{m}_{hf}") for hf in range(2)]
                for m in range(4)]
        v_sb = [io.tile([128, HPC * 65], BF16, name=f"v_sb{t}") for t in range(NKT)]
        for t in range(NKT):
            v3 = v_sb[t][:].rearrange("p (h c) -> p h c", c=65)
            nc.vector.memset(v3[:, :, 64:65], 1.0)

        def qk_proj(m, half, w_sb, dst_tiles):
            """project m-tile (heads 2m,2m+1), qi/ki half -> rope -> bf16.
            Emitted in two 512-column chunks on single-bank psum slots so
            the rope evacuation of chunk 0 overlaps chunk 1's matmuls."""
            dst = dst_tiles[m][half]
            for c in range(2):
                ps = prp.tile([128, 512], F32, tag="pj", name="ps_proj")
                for k in range(8):
                    lhsT = w_sb[:, 512 * k + 128 * m : 512 * k + 128 * m + 128]
                    nc.tensor.matmul(
                        ps[:],
                        lhsT,
                        xt_at(k, HALF * half + 512 * c, 512),
                        start=(k == 0),
                        stop=(k == 7),
                    )
                hs = slice(HALF * half + 512 * c, HALF * half + 512 * (c + 1))
                ds = slice(512 * c, 512 * (c + 1))
                p1 = rp.tile([128, 512], BF16, tag="p1")
                nc.vector.tensor_mul(p1[:], ps[:], cosT[:, hs])
                p2 = rp.tile([128, 512], BF16, tag="p2")
                nc.vector.tensor_mul(p2[:], ps[:], sinT[:, hs])
                q2 = rp.tile([128, 512], BF16, tag="q2")
                nc.vector.stream_shuffle(q2[:], p2[:], SHUF)
                nc.vector.scalar_tensor_tensor(
                    dst[:, ds], q2[:], sign[:], p1[:], ALU.mult, ALU.add
                )

        def v_proj(kb):
            ps_v = prp.tile([128, DH], F32, tag="pj", name="ps_v")
            for k in range(8):
                nc.tensor.matmul(
                    ps_v[:],
                    xt_at(k, 128 * kb, 128),
                    wv[:, 512 * k : 512 * (k + 1)],
                    start=(k == 0),
                    stop=(k == 7),
                )
            v3 = v_sb[kb][:].rearrange("p (h c) -> p h c", c=65)
            nc.vector.tensor_copy(
                v3[:, :, 0:64], ps_v[:].rearrange("p (h c) -> p h c", c=64)
            )

        def attn_scores(h, half, t):
            """one score tile + exp; returns the es tile."""
            m, o = h // 2, 64 * (h % 2)
            kt = k_sb[m][t // 8]
            ko = 128 * (t % 8)
            st = pp.tile([128, HALF], F32, tag="st", name="ps_st")
            for c in range(2):
                nc.tensor.matmul(
                    st[:, 512 * c : 512 * (c + 1)],
                    kt[o : o + 64, ko : ko + 128],
                    q_sb[m][half][o : o + 64, 512 * c : 512 * (c + 1)],
                    start=True,
                    stop=True,
                )
            es = esp.tile([128, HALF], BF16, tag="es")
            nc.scalar.activation(
                es[:], st[:], AF.Exp,
                bias=maskT[:, t : t + 1], scale=0.125,
            )
            return es

        def attn_pv(h, otc, t, es):
            for c in range(2):
                nc.tensor.matmul(
                    otc[c][:],
                    v_sb[t][:, 65 * h : 65 * h + 65],
                    es[:, 512 * c : 512 * (c + 1)],
                    start=(t == 0),
                    stop=(t == NKT - 1),
                )

        def attn_epilogue(h, half, otc):
            m, o = h // 2, 64 * (h % 2)
            for c in range(2):
                rec = mis.tile([1, 512], F32, tag="rec")
                nc.vector.reciprocal(rec[:], otc[c][64:65, :])
                bcs = mis.tile([64, 512], F32, tag="bcs")
                nc.gpsimd.partition_broadcast(bcs[:], rec[0:1, :], channels=64)
                nc.vector.scalar_tensor_tensor(
                    o_sb[m][half][o : o + 64, 512 * c : 512 * (c + 1)],
                    otc[c][0:64, :], 1.0, bcs[:], ALU.mult, ALU.mult,
                )

        def attn_phase(cur, prev, fillers=()):
            """Pipeline phase: slot `cur`=(h,half) scores+exp, interleaved
            with slot `prev`'s PVs (from its saved es tiles) + fillers.
            Returns cur's state for the next phase."""
            ess = []
            if prev is not None:
                ph, phalf, pess = prev
                potc = [
                    otp.tile([65, 512], F32, tag="ot", name=f"ot{c}")
                    for c in range(2)
                ]
            fillers = dict(fillers)
            for t in range(NKT):
                if cur is not None:
                    ess.append(attn_scores(cur[0], cur[1], t))
                if t in fillers:
                    fillers[t]()
                if prev is not None:
                    attn_pv(ph, potc, t, pess[t])
            if prev is not None:
                attn_epilogue(ph, phalf, potc)
            if cur is None:
                return None
            return (cur[0], cur[1], ess)

        def outproj_qb(qb, tags=("pj", "pj")):
            hf, qo = qb // 8, 128 * (qb % 8)
            for c in range(2):
                pool = pp if tags[c] == "st" else prp
                po = pool.tile([128, 512], F32, tag=tags[c], name="ps_po")
                for dt_ in range(4):
                    nc.tensor.matmul(
                        po[:],
                        o_sb[dt_][hf][:, qo : qo + 128],
                        wo[:, D * dt_ + 512 * c : D * dt_ + 512 * (c + 1)],
                        start=(dt_ == 0),
                        stop=(dt_ == 3),
                    )
                ob = obp.tile([128, 512], BF16, tag="ob")
                nc.vector.tensor_copy(ob[:], po[:])
                nc.sync.dma_start(
                    out_d[128 * qb : 128 * (qb + 1), 512 * c : 512 * (c + 1)],
                    ob[:],
                )

        def qk_group(m):
            qk_proj(m, 0, wq, q_sb)
            qk_proj(m, 0, wk, k_sb)
            qk_proj(m, 1, wk, k_sb)

        for _rep in range(repeats):
            qk_group(0)
            s = attn_phase(
                (0, 0), None,
                fillers={t: (lambda kb=t: v_proj(kb)) for t in range(NKT)},
            )
            qk_group(1)
            s = attn_phase((1, 0), s)
            qk_group(2)
            s = attn_phase((2, 0), s)
            qk_group(3)
            s = attn_phase((3, 0), s)
            s = attn_phase((4, 0), s)
            s = attn_phase((5, 0), s)
            qk_proj(0, 1, wq, q_sb)
            s = attn_phase((6, 0), s)
            qk_proj(1, 1, wq, q_sb)
            s = attn_phase((7, 0), s)
            qk_proj(2, 1, wq, q_sb)
            s = attn_phase((0, 1), s)
            qk_proj(3, 1, wq, q_sb)
            # half-1 phases; outproj of half-0 qi blocks as fillers
            qbf = [[0], [1], [2], [3], [4, 5], [6, 7]]
            for i, h in enumerate(range(1, 7)):
                fl = {
                    4 + 8 * j: (lambda qb=qb: outproj_qb(qb))
                    for j, qb in enumerate(qbf[i])
                }
                s = attn_phase((h, 1), s, fillers=fl)
            s = attn_phase((7, 1), s)
            attn_phase(None, s)
            for qb in range(8, NQB):
                outproj_qb(qb, tags=("st", "pj"))
    nc.compile()
    return nc


def _prep_core_inputs(x, cosT, sinT, mask, W_q, W_k, W_v, W_o, b, g):
    bf = ml_dtypes.bfloat16
    gs = slice(g * DH, (g + 1) * DH)

    # RoPE interleave row permutation within the head-group slice:
    # per head: [x0(0:16), x1(0:16), x0(16:32), x1(16:32)]
    j = np.arange(64)
    blk, within = j // 16, j % 16
    perm64 = np.where(
        blk == 0, 2 * within,
        np.where(blk == 1, 2 * within + 1,
                 np.where(blk == 2, 2 * within + 32, 2 * within + 33)),
    )
    perm = (np.arange(HPC)[:, None] * 64 + perm64[None, :]).reshape(-1) + g * DH

    def wtile(wT):  # [1024, 512] -> [128, 8*512] (k-tile k at cols 512k)
        return np.ascontiguousarray(
            wT.reshape(8, 128, DH).transpose(1, 0, 2).reshape(128, 8 * DH)
        ).astype(bf)

    xt = np.ascontiguousarray(
        x[b].T.reshape(8, 128, L).transpose(1, 0, 2).reshape(128, 8 * L)
    ).astype(bf)
    wq = wtile(W_q[perm].T)
    wk = wtile(W_k[perm].T)
    wv = wtile(W_v[gs].T)
    wo = np.ascontiguousarray(
        W_o[:, gs].T.reshape(4, 128, D).transpose(1, 0, 2).reshape(128, 4 * D)
    ).astype(bf)
    return {
        "xt": xt, "wq": wq, "wk": wk, "wv": wv, "wo": wo,
        "cosT": cosT, "sinT": sinT,
        "maskT": np.ascontiguousarray(mask[b].reshape(NKT, 128).T).astype(
            np.float32
        ),
    }


def make_in_maps(x, freqs_cos, freqs_sin, attention_mask, W_q, W_k, W_v, W_o):
    bf = ml_dtypes.bfloat16
    x = np.asarray(x, np.float32)
    # cos/sin rows follow the rope row layout: r%64 -> dim (r%16) + 16*((r%64)//32)
    r = np.arange(128)
    dim_idx = (r % 16) + 16 * ((r % 64) // 32)
    cosT = np.ascontiguousarray(
        np.asarray(freqs_cos, np.float32).T[dim_idx]
    ).astype(bf)
    sinT = np.ascontiguousarray(
        np.asarray(freqs_sin, np.float32).T[dim_idx]
    ).astype(bf)
    mask = np.asarray(attention_mask, np.float32)
    W_q, W_k = np.asarray(W_q, np.float32), np.asarray(W_k, np.float32)
    W_v, W_o = np.asarray(W_v, np.float32), np.asarray(W_o, np.float32)
    return [
        _prep_core_inputs(x, cosT, sinT, mask, W_q, W_k, W_v, W_o, c // 2, c % 2)
        for c in range(8)
    ]


_CACHE = {}


def kernel(x, freqs_cos, freqs_sin, attention_mask, W_q, W_k, W_v, W_o):
    from concourse.bass_utils import run_bass_kernel_spmd

    if "nc" not in _CACHE:
        _CACHE["nc"] = build_nc()
    nc = _CACHE["nc"]
    in_maps = make_in_maps(
        x, freqs_cos, freqs_sin, attention_mask, W_q, W_k, W_v, W_o
    )
    res = run_bass_kernel_spmd(nc, in_maps, core_ids=list(range(8)))
    outs = [np.asarray(r["out"], np.float32) for r in res.results]
    full = np.stack([outs[2 * b] + outs[2 * b + 1] for b in range(B)], axis=0)
    return full.astype(np.float32)


if __name__ == "__main__":
    nc = build_nc()
    print("built ok")
